# revision 26
# baseline (speedup 1.0000x reference)
"""Trainium2 Bass kernel for nn_AttentionFlow (T=8192, J=1024, D=256, 8 cores).

Reference math:
  w_c, w_q, w_m = w[:D], w[D:2D], w[2D:]
  S[t,j] = ctx@w_c [t] + q@w_q [j] + (ctx*w_m) @ q.T     [T, J]
  A = softmax_j(S);  c2q = A @ q                          [T, D]
  b = max_j S;       h = b @ ctx                          [D]
  G = [ctx, c2q, ctx*c2q, ctx*h]                          [T, 4D]

Sharding: rows (t) split across 8 cores, 1024 rows each. Only h needs an
AllReduce ([256] floats). qwq = q@w_q ([J]) is precomputed host-side (0.5
MFLOP) and passed as an input.

Per-core structure (bf16 matmuls, f32 PSUM accumulation, no softmax
max-subtraction needed since |S| <= ~6):

  prep:    Qm = q * w_m; QmT via PE transposes.
  phase A (per 128-row t-tile): ctxT via PE transpose; S = qwq-seed (K=1
           matmul) + ctx @ Qm.T in PSUM; rowmax -> m; cwc = ctx @ w_c
           (2 small matmuls); b = m + cwc.  No exp here.
  h:       h = sum_t b_t ctx_t via matmul, then AllReduce [256] launches
           early and overlaps phases B/C.
  phase B (per 128-col j-chunk): S.T = Qm @ ctx.T via matmuls (same
           operands swapped -> E.T comes out of exp already transposed;
           qwq enters as the per-partition activation bias).
  phase C (per t-tile): U = E.T-chunks.T @ [q | 1]; ones column gives the
           softmax denominators; c2q = U[:, :D] / U[:, D]; assemble
           [ctx, c2q, ctx*c2q] in one SBUF tile, single 3KB-row DMA out.
  phase D (per t-tile): G4 = ctx * h_bcast after the AllReduce lands.
"""

import sys

if "/opt/trn_rl_repo" not in sys.path:
    sys.path.insert(0, "/opt/trn_rl_repo")

import numpy as np

import concourse.bass as bass
import concourse.bacc as bacc
import concourse.tile as tile
from concourse import mybir
from concourse.bass_utils import run_bass_kernel_spmd
from concourse.masks import make_identity

T, J, D = 8192, 1024, 256
N_CORES = 8
T_LOC = T // N_CORES          # 1024 rows per core
NT = T_LOC // 128             # 8 t-tiles per core
NJ = J // 128                 # 8 j-chunks
F32 = mybir.dt.float32
BF16 = mybir.dt.bfloat16


def _build_program():
    nc = bacc.Bacc("TRN2", target_bir_lowering=False, debug=False,
                   num_devices=N_CORES)
    ctx_ap = nc.dram_tensor("context", [T_LOC, D], F32, kind="ExternalInput").ap()
    q_ap = nc.dram_tensor("query", [J, D], F32, kind="ExternalInput").ap()
    w_ap = nc.dram_tensor("w", [3 * D], F32, kind="ExternalInput").ap()
    qwq_ap = nc.dram_tensor("qwq", [J], F32, kind="ExternalInput").ap()
    out_ap = nc.dram_tensor("out", [T_LOC, 4 * D], F32, kind="ExternalOutput").ap()
    warm_ap = nc.dram_tensor("warm", [128, 1], F32, kind="ExternalOutput").ap()

    with tile.TileContext(nc) as tc:
        _emit(tc, out_ap, ctx_ap, q_ap, w_ap, qwq_ap, warm_ap)
        tc._emit_exitstack.close()
    nc.compile()
    return nc


def _emit(tc, out_ap, ctx_ap, q_ap, w_ap, qwq_ap, warm_ap):
    from contextlib import ExitStack
    nc = tc.nc
    AF = mybir.ActivationFunctionType
    ALU = mybir.AluOpType

    es = ExitStack()
    tc._emit_exitstack = es
    singles = es.enter_context(tc.tile_pool(name="singles", bufs=1))
    wk_sm = es.enter_context(tc.tile_pool(name="wk_sm", bufs=4))
    wk_g = es.enter_context(tc.tile_pool(name="wk_g", bufs=3))
    ps_S = es.enter_context(tc.tile_pool(name="ps_S", bufs=3, space="PSUM"))
    ps_TC = es.enter_context(tc.tile_pool(name="ps_TC", bufs=2, space="PSUM"))
    ps_U = es.enter_context(tc.tile_pool(name="ps_U", bufs=2, space="PSUM"))
    ps_h = es.enter_context(tc.tile_pool(name="ps_h", bufs=1, space="PSUM"))
    dram = es.enter_context(tc.tile_pool(name="dram", bufs=1, space="DRAM"))

    # ---------------- one-time prep ----------------
    ident = singles.tile([128, 128], BF16)
    make_identity(nc, ident)

    # PE warm-up spin: dense dummy matmuls while the input DMAs run, so the
    # HAM clock gate releases (1.2 -> 2.4 GHz) before the real matmuls start.
    # The result is sunk to a tiny output so nothing dead-code-eliminates it.
    warm_src = singles.tile([128, 512], BF16)
    nc.vector.memset(warm_src, 0.001)
    wps = None
    for i in range(20):
        wps = ps_S.tile([128, 512], F32, tag="S")
        nc.tensor.matmul(wps, warm_src[:, 0:128], warm_src, start=True,
                         stop=True)
    warm_sb = singles.tile([128, 1], F32)
    nc.vector.reduce_max(warm_sb, wps, axis=mybir.AxisListType.X)
    nc.sync.dma_start(out=warm_ap, in_=warm_sb)

    # query, natural layout [p, jc, d]  (j = jc*128 + p)
    q_f32 = singles.tile([128, NJ, D], F32)
    nc.sync.dma_start(out=q_f32, in_=q_ap.rearrange("(c p) d -> p c d", p=128))

    # w_m broadcast across partitions; qwq in partition-major column form
    wm_bc = singles.tile([128, D], F32)
    nc.sync.dma_start(
        out=wm_bc,
        in_=w_ap[2 * D:3 * D].rearrange("(a d) -> a d", a=1).to_broadcast([128, D]))
    qwqT = singles.tile([128, NJ], F32)
    nc.sync.dma_start(out=qwqT, in_=qwq_ap.rearrange("(c p) -> p c", p=128))
    # qwq row (bf16) for the K=1 PSUM seed matmul in phase A
    qwq_bf = singles.tile([1, J], BF16)
    nc.gpsimd.dma_start(out=qwq_bf, in_=qwq_ap.rearrange("(a d) -> a d", a=1))
    ones_bf = singles.tile([1, 128], BF16)
    nc.vector.memset(ones_bf, 1.0)
    # w_c in partition-major form for the cwc matmuls
    wc_pm = singles.tile([128, 2], F32)
    nc.sync.dma_start(out=wc_pm, in_=w_ap[0:D].rearrange("(c p) -> p c", p=128))
    wc_pm_bf = singles.tile([128, 2], BF16)
    nc.scalar.copy(wc_pm_bf, wc_pm)

    # q_aug: bf16 [q | 1] moving operand of the U matmuls
    q_aug = singles.tile([128, NJ, D + 1], BF16)
    qm_bf = singles.tile([128, NJ, D], BF16)
    for jc in range(NJ):
        nc.scalar.copy(q_aug[:, jc, 0:D], q_f32[:, jc, :])
        nc.vector.tensor_mul(qm_bf[:, jc, :], q_f32[:, jc, :], wm_bc)
    nc.vector.memset(q_aug[:, :, D:D + 1], 1.0)

    # QmT [d-partitions, dc, j] via PE transposes
    QmT = singles.tile([128, 2, J], BF16)
    for jc in range(NJ):
        for dc in range(2):
            pt = ps_TC.tile([128, 128], BF16, tag="T")
            nc.tensor.transpose(pt, qm_bf[:, jc, dc * 128:(dc + 1) * 128], ident)
            nc.vector.tensor_copy(QmT[:, dc, jc * 128:(jc + 1) * 128], pt)

    # persistent per-core state
    ctx_f32 = singles.tile([128, NT, D], F32)
    ctx_bf = singles.tile([128, NT, D], BF16)
    ctxT_all = singles.tile([128, 2, T_LOC], BF16)
    ET_all = singles.tile([128, NJ, T_LOC], BF16)
    b_all = singles.tile([128, NT], BF16)
    h_bc = singles.tile([128, D], F32)

    # ---------------- prologue: load + transpose all ctx tiles ----------------
    for t in range(NT):
        rows = slice(t * 128, (t + 1) * 128)
        nc.sync.dma_start(out=ctx_f32[:, t, :], in_=ctx_ap[rows, :])
        nc.scalar.copy(ctx_bf[:, t, :], ctx_f32[:, t, :])
        for dc in range(2):
            pt = ps_TC.tile([128, 128], BF16, tag="T")
            nc.tensor.transpose(pt, ctx_bf[:, t, dc * 128:(dc + 1) * 128], ident)
            nc.vector.tensor_copy(ctxT_all[:, dc, t * 128:(t + 1) * 128], pt)

    # ---------------- phase A: S row-maxes, b, h accumulation ----------------
    ph = ps_h.tile([1, D], F32)
    for t in range(NT):
        m = wk_sm.tile([128, 3], F32)
        for jh in range(2):
            ps = ps_S.tile([128, 512], F32, tag="S")
            nc.tensor.matmul(ps, ones_bf, qwq_bf[:, jh * 512:(jh + 1) * 512],
                             start=True, stop=False)
            for dc in range(2):
                nc.tensor.matmul(
                    ps, ctxT_all[:, dc, t * 128:(t + 1) * 128],
                    QmT[:, dc, jh * 512:(jh + 1) * 512],
                    start=False, stop=(dc == 1))
            nc.vector.reduce_max(m[:, jh:jh + 1], ps, axis=mybir.AxisListType.X)
        nc.vector.tensor_max(m[:, 0:1], m[:, 0:1], m[:, 1:2])

        ps_c = ps_TC.tile([128, 1], F32, tag="T")
        for dc in range(2):
            nc.tensor.matmul(ps_c, ctxT_all[:, dc, t * 128:(t + 1) * 128],
                             wc_pm_bf[:, dc:dc + 1],
                             start=(dc == 0), stop=(dc == 1))
        nc.vector.tensor_add(b_all[:, t:t + 1], m[:, 0:1], ps_c)
        nc.tensor.matmul(ph, b_all[:, t:t + 1], ctx_bf[:, t, :],
                         start=(t == 0), stop=(t == NT - 1),
                         skip_group_check=True)

    # ---------------- h AllReduce (overlaps B/C) ----------------
    h_sb = singles.tile([1, D], F32)
    nc.scalar.copy(h_sb, ph)
    h_in = dram.tile([1, D], F32)
    h_out = dram.tile([1, D], F32)
    nc.scalar.dma_start(out=h_in, in_=h_sb)
    nc.gpsimd.collective_compute(
        "AllReduce",
        ALU.add,
        replica_groups=[list(range(N_CORES))],
        ins=[h_in.opt()],
        outs=[h_out.opt()])
    nc.sync.dma_start(out=h_bc, in_=h_out.to_broadcast([128, D]))

    # ---------------- phase B: E.T per j-chunk ----------------
    for jc in range(NJ):
        for th in range(2):
            ps = ps_S.tile([128, 512], F32, tag="S")
            for dc in range(2):
                nc.tensor.matmul(
                    ps, QmT[:, dc, jc * 128:(jc + 1) * 128],
                    ctxT_all[:, dc, th * 512:(th + 1) * 512],
                    start=(dc == 0), stop=(dc == 1))
            nc.scalar.activation(ET_all[:, jc, th * 512:(th + 1) * 512], ps,
                                 AF.Exp, bias=qwqT[:, jc:jc + 1])

    # ---------------- phase C: U, c2q, G[:, 0:768] per t-tile ----------------
    for t in range(NT):
        rows = slice(t * 128, (t + 1) * 128)
        pu = ps_U.tile([128, D + 1], F32, tag="U")
        for jc in range(NJ):
            nc.tensor.matmul(pu, ET_all[:, jc, t * 128:(t + 1) * 128],
                             q_aug[:, jc, :],
                             start=(jc == 0), stop=(jc == NJ - 1))
        r = wk_sm.tile([128, 1], F32, tag="recip")
        nc.vector.reciprocal(r, pu[:, D:D + 1])
        g123 = wk_g.tile([128, 3 * D], F32, tag="g123")
        nc.vector.tensor_copy(g123[:, 0:D], ctx_f32[:, t, :])
        nc.scalar.activation(g123[:, D:2 * D], pu[:, 0:D], AF.Copy, scale=r)
        nc.vector.tensor_mul(g123[:, 2 * D:3 * D], ctx_f32[:, t, :],
                             g123[:, D:2 * D])
        nc.scalar.dma_start(out=out_ap[rows, 0:3 * D], in_=g123)

    # ---------------- phase D: G4 after the AllReduce ----------------
    for t in range(NT):
        rows = slice(t * 128, (t + 1) * 128)
        g4 = wk_g.tile([128, D], F32, tag="g4")
        nc.vector.tensor_mul(g4, ctx_f32[:, t, :], h_bc)
        nc.scalar.dma_start(out=out_ap[rows, 3 * D:4 * D], in_=g4)


_NC_CACHE = None


def _get_program():
    global _NC_CACHE
    if _NC_CACHE is None:
        _NC_CACHE = _build_program()
    return _NC_CACHE


def kernel(context: np.ndarray, query: np.ndarray, w: np.ndarray,
           **kwargs) -> np.ndarray:
    context = np.ascontiguousarray(context, dtype=np.float32)
    query = np.ascontiguousarray(query, dtype=np.float32)
    w = np.ascontiguousarray(w, dtype=np.float32)
    qwq = query @ w[D:2 * D]

    nc = _get_program()
    shard = T_LOC
    in_maps = [
        {
            "context": context[i * shard:(i + 1) * shard],
            "query": query,
            "w": w,
            "qwq": qwq,
        }
        for i in range(N_CORES)
    ]
    res = run_bass_kernel_spmd(nc, in_maps, core_ids=list(range(N_CORES)))
    return np.concatenate([res.results[i]["out"] for i in range(N_CORES)],
                          axis=0)


# revision 27
# speedup vs baseline: 1.2170x; 1.2170x over previous
"""Trainium2 Bass kernel for nn_AttentionFlow (T=8192, J=1024, D=256, 8 cores).

Reference math:
  w_c, w_q, w_m = w[:D], w[D:2D], w[2D:]
  S[t,j] = ctx@w_c [t] + q@w_q [j] + (ctx*w_m) @ q.T     [T, J]
  A = softmax_j(S);  c2q = A @ q                          [T, D]
  b = max_j S;       h = b @ ctx                          [D]
  G = [ctx, c2q, ctx*c2q, ctx*h]                          [T, 4D]

Sharding: rows (t) split across 8 cores, 1024 rows each. Only h needs an
AllReduce ([256] floats). qwq = q@w_q ([J]) is precomputed host-side (0.5
MFLOP) and passed as an input.

Per-core structure (bf16 matmuls, f32 PSUM accumulation, no softmax
max-subtraction needed since |S| <= ~6):

  prep:    Qm = q * w_m; QmT via PE transposes.
  phase A (per 128-row t-tile): ctxT via PE transpose; S = qwq-seed (K=1
           matmul) + ctx @ Qm.T in PSUM; rowmax -> m; cwc = ctx @ w_c
           (2 small matmuls); b = m + cwc.  No exp here.
  h:       h = sum_t b_t ctx_t via matmul, then AllReduce [256] launches
           early and overlaps phases B/C.
  phase B (per 128-col j-chunk): S.T = Qm @ ctx.T via matmuls (same
           operands swapped -> E.T comes out of exp already transposed;
           qwq enters as the per-partition activation bias).
  phase C (per t-tile): U = E.T-chunks.T @ [q | 1]; ones column gives the
           softmax denominators; c2q = U[:, :D] / U[:, D]; assemble
           [ctx, c2q, ctx*c2q] in one SBUF tile, single 3KB-row DMA out.
  phase D (per t-tile): G4 = ctx * h_bcast after the AllReduce lands.
"""

import sys

if "/opt/trn_rl_repo" not in sys.path:
    sys.path.insert(0, "/opt/trn_rl_repo")

import numpy as np

import concourse.bass as bass
import concourse.bacc as bacc
import concourse.tile as tile
from concourse import mybir
from concourse.bass_utils import run_bass_kernel_spmd
from concourse.masks import make_identity

T, J, D = 8192, 1024, 256
N_CORES = 8
T_LOC = T // N_CORES          # 1024 rows per core
NT = T_LOC // 128             # 8 t-tiles per core
NJ = J // 128                 # 8 j-chunks
F32 = mybir.dt.float32
BF16 = mybir.dt.bfloat16


def _build_program():
    nc = bacc.Bacc("TRN2", target_bir_lowering=False, debug=False,
                   num_devices=N_CORES)
    ctx_ap = nc.dram_tensor("context", [T_LOC, D], F32, kind="ExternalInput").ap()
    q_ap = nc.dram_tensor("query", [J, D], F32, kind="ExternalInput").ap()
    w_ap = nc.dram_tensor("w", [3 * D], F32, kind="ExternalInput").ap()
    qwq_ap = nc.dram_tensor("qwq", [J], F32, kind="ExternalInput").ap()
    out_ap = nc.dram_tensor("out", [T_LOC, 4 * D], F32, kind="ExternalOutput").ap()
    warm_ap = nc.dram_tensor("warm", [128, 1], F32, kind="ExternalOutput").ap()

    with tile.TileContext(nc) as tc:
        _emit(tc, out_ap, ctx_ap, q_ap, w_ap, qwq_ap, warm_ap)
        tc._emit_exitstack.close()
    nc.compile()
    return nc


def _emit(tc, out_ap, ctx_ap, q_ap, w_ap, qwq_ap, warm_ap):
    from contextlib import ExitStack
    nc = tc.nc
    AF = mybir.ActivationFunctionType
    ALU = mybir.AluOpType

    es = ExitStack()
    tc._emit_exitstack = es
    singles = es.enter_context(tc.tile_pool(name="singles", bufs=1))
    wk_sm = es.enter_context(tc.tile_pool(name="wk_sm", bufs=4))
    wk_g = es.enter_context(tc.tile_pool(name="wk_g", bufs=3))
    ps_S = es.enter_context(tc.tile_pool(name="ps_S", bufs=3, space="PSUM"))
    ps_TC = es.enter_context(tc.tile_pool(name="ps_TC", bufs=2, space="PSUM"))
    ps_U = es.enter_context(tc.tile_pool(name="ps_U", bufs=2, space="PSUM"))
    ps_h = es.enter_context(tc.tile_pool(name="ps_h", bufs=1, space="PSUM"))
    dram = es.enter_context(tc.tile_pool(name="dram", bufs=1, space="DRAM"))

    # ---------------- one-time prep ----------------
    ident = singles.tile([128, 128], BF16)
    make_identity(nc, ident)

    # PE warm-up spin: dense dummy matmuls while the input DMAs run, so the
    # HAM clock gate releases (1.2 -> 2.4 GHz) before the real matmuls start.
    # The result is sunk to a tiny output so nothing dead-code-eliminates it.
    warm_src = singles.tile([128, 512], BF16)
    nc.vector.memset(warm_src, 0.001)
    wps = None
    for i in range(24):
        wps = ps_S.tile([128, 512], F32, tag="S")
        nc.tensor.matmul(wps, warm_src[:, 0:128], warm_src, start=True,
                         stop=True)
    warm_sb = singles.tile([128, 1], F32)
    nc.vector.reduce_max(warm_sb, wps, axis=mybir.AxisListType.X)
    nc.sync.dma_start(out=warm_ap, in_=warm_sb)

    # query, natural layout [p, jc, d]  (j = jc*128 + p)
    q_f32 = singles.tile([128, NJ, D], F32)
    nc.sync.dma_start(out=q_f32, in_=q_ap.rearrange("(c p) d -> p c d", p=128))

    # w_m broadcast across partitions; qwq in partition-major column form
    wm_bc = singles.tile([128, D], F32)
    nc.sync.dma_start(
        out=wm_bc,
        in_=w_ap[2 * D:3 * D].rearrange("(a d) -> a d", a=1).to_broadcast([128, D]))
    qwqT = singles.tile([128, NJ], F32)
    nc.sync.dma_start(out=qwqT, in_=qwq_ap.rearrange("(c p) -> p c", p=128))
    # qwq row (bf16) for the K=1 PSUM seed matmul in phase A
    qwq_bf = singles.tile([1, J], BF16)
    nc.gpsimd.dma_start(out=qwq_bf, in_=qwq_ap.rearrange("(a d) -> a d", a=1))
    ones_bf = singles.tile([1, 128], BF16)
    nc.vector.memset(ones_bf, 1.0)
    # w_c in partition-major form for the cwc matmuls
    wc_pm = singles.tile([128, 2], F32)
    nc.sync.dma_start(out=wc_pm, in_=w_ap[0:D].rearrange("(c p) -> p c", p=128))
    wc_pm_bf = singles.tile([128, 2], BF16)
    nc.scalar.copy(wc_pm_bf, wc_pm)

    # q_aug: bf16 [q | 1] moving operand of the U matmuls
    q_aug = singles.tile([128, NJ, D + 1], BF16)
    qm_bf = singles.tile([128, NJ, D], BF16)
    for jc in range(NJ):
        nc.scalar.copy(q_aug[:, jc, 0:D], q_f32[:, jc, :])
        nc.vector.tensor_mul(qm_bf[:, jc, :], q_f32[:, jc, :], wm_bc)
    nc.vector.memset(q_aug[:, :, D:D + 1], 1.0)

    # QmT [d-partitions, dc, j] via PE transposes
    QmT = singles.tile([128, 2, J], BF16)
    for jc in range(NJ):
        for dc in range(2):
            pt = ps_TC.tile([128, 128], BF16, tag="T")
            nc.tensor.transpose(pt, qm_bf[:, jc, dc * 128:(dc + 1) * 128], ident)
            nc.vector.tensor_copy(QmT[:, dc, jc * 128:(jc + 1) * 128], pt)

    # persistent per-core state
    ctx_f32 = singles.tile([128, NT, D], F32)
    ctx_bf = singles.tile([128, NT, D], BF16)
    ctxT_all = singles.tile([128, 2, T_LOC], BF16)
    ET_all = singles.tile([128, NJ, T_LOC], BF16)
    b_all = singles.tile([128, NT], BF16)
    h_bc = singles.tile([128, D], F32)

    # ---------------- prologue: load + transpose all ctx tiles ----------------
    for t in range(NT):
        rows = slice(t * 128, (t + 1) * 128)
        nc.sync.dma_start(out=ctx_f32[:, t, :], in_=ctx_ap[rows, :])
        nc.scalar.copy(ctx_bf[:, t, :], ctx_f32[:, t, :])
        for dc in range(2):
            pt = ps_TC.tile([128, 128], BF16, tag="T")
            nc.tensor.transpose(pt, ctx_bf[:, t, dc * 128:(dc + 1) * 128], ident)
            nc.vector.tensor_copy(ctxT_all[:, dc, t * 128:(t + 1) * 128], pt)

    # ---------------- phase A: S row-maxes, b, h accumulation ----------------
    ph = ps_h.tile([1, D], F32)
    for t in range(NT):
        m = wk_sm.tile([128, 3], F32)
        for jh in range(2):
            ps = ps_S.tile([128, 512], F32, tag="S")
            nc.tensor.matmul(ps, ones_bf, qwq_bf[:, jh * 512:(jh + 1) * 512],
                             start=True, stop=False)
            for dc in range(2):
                nc.tensor.matmul(
                    ps, ctxT_all[:, dc, t * 128:(t + 1) * 128],
                    QmT[:, dc, jh * 512:(jh + 1) * 512],
                    start=False, stop=(dc == 1))
            nc.vector.reduce_max(m[:, jh:jh + 1], ps, axis=mybir.AxisListType.X)
        nc.vector.tensor_max(m[:, 0:1], m[:, 0:1], m[:, 1:2])

        ps_c = ps_TC.tile([128, 1], F32, tag="T")
        for dc in range(2):
            nc.tensor.matmul(ps_c, ctxT_all[:, dc, t * 128:(t + 1) * 128],
                             wc_pm_bf[:, dc:dc + 1],
                             start=(dc == 0), stop=(dc == 1))
        nc.vector.tensor_add(b_all[:, t:t + 1], m[:, 0:1], ps_c)
        nc.tensor.matmul(ph, b_all[:, t:t + 1], ctx_bf[:, t, :],
                         start=(t == 0), stop=(t == NT - 1),
                         skip_group_check=True)

    # ---------------- h AllReduce (overlaps B/C) ----------------
    h_sb = singles.tile([1, D], F32)
    nc.scalar.copy(h_sb, ph)
    h_in = dram.tile([1, D], F32)
    h_out = dram.tile([1, D], F32)
    nc.scalar.dma_start(out=h_in, in_=h_sb)
    nc.gpsimd.collective_compute(
        "AllReduce",
        ALU.add,
        replica_groups=[list(range(N_CORES))],
        ins=[h_in.opt()],
        outs=[h_out.opt()])
    nc.sync.dma_start(out=h_bc, in_=h_out.to_broadcast([128, D]))

    # ---------------- phase B: E.T per j-chunk ----------------
    for jc in range(NJ):
        for th in range(2):
            ps = ps_S.tile([128, 512], F32, tag="S")
            for dc in range(2):
                nc.tensor.matmul(
                    ps, QmT[:, dc, jc * 128:(jc + 1) * 128],
                    ctxT_all[:, dc, th * 512:(th + 1) * 512],
                    start=(dc == 0), stop=(dc == 1))
            nc.scalar.activation(ET_all[:, jc, th * 512:(th + 1) * 512], ps,
                                 AF.Exp, bias=qwqT[:, jc:jc + 1])

    # ---------------- phase C: U, c2q, G[:, 0:768] per t-tile ----------------
    for t in range(NT):
        rows = slice(t * 128, (t + 1) * 128)
        pu = ps_U.tile([128, D + 1], F32, tag="U")
        for jc in range(NJ):
            nc.tensor.matmul(pu, ET_all[:, jc, t * 128:(t + 1) * 128],
                             q_aug[:, jc, :],
                             start=(jc == 0), stop=(jc == NJ - 1))
        r = wk_sm.tile([128, 1], F32, tag="recip")
        nc.vector.reciprocal(r, pu[:, D:D + 1])
        g123 = wk_g.tile([128, 3 * D], F32, tag="g123")
        nc.vector.tensor_copy(g123[:, 0:D], ctx_f32[:, t, :])
        nc.scalar.activation(g123[:, D:2 * D], pu[:, 0:D], AF.Copy, scale=r)
        nc.vector.tensor_mul(g123[:, 2 * D:3 * D], ctx_f32[:, t, :],
                             g123[:, D:2 * D])
        nc.scalar.dma_start(out=out_ap[rows, 0:3 * D], in_=g123)

    # ---------------- phase D: G4 after the AllReduce ----------------
    for t in range(NT):
        rows = slice(t * 128, (t + 1) * 128)
        g4 = wk_g.tile([128, D], F32, tag="g4")
        nc.vector.tensor_mul(g4, ctx_f32[:, t, :], h_bc)
        nc.scalar.dma_start(out=out_ap[rows, 3 * D:4 * D], in_=g4)


_NC_CACHE = None


def _get_program():
    global _NC_CACHE
    if _NC_CACHE is None:
        _NC_CACHE = _build_program()
    return _NC_CACHE


def kernel(context: np.ndarray, query: np.ndarray, w: np.ndarray,
           **kwargs) -> np.ndarray:
    context = np.ascontiguousarray(context, dtype=np.float32)
    query = np.ascontiguousarray(query, dtype=np.float32)
    w = np.ascontiguousarray(w, dtype=np.float32)
    qwq = query @ w[D:2 * D]

    nc = _get_program()
    shard = T_LOC
    in_maps = [
        {
            "context": context[i * shard:(i + 1) * shard],
            "query": query,
            "w": w,
            "qwq": qwq,
        }
        for i in range(N_CORES)
    ]
    res = run_bass_kernel_spmd(nc, in_maps, core_ids=list(range(N_CORES)))
    return np.concatenate([res.results[i]["out"] for i in range(N_CORES)],
                          axis=0)
